# revision 1
# baseline (speedup 1.0000x reference)
"""Trainium2 Bass kernel for nn_EuclideanExperts (8-expert 2-layer GraphSAGE).

Expert-parallel: each of the 8 NeuronCores runs one expert's full encoder.
The graph aggregation (mean over in-neighbors) is computed as a sequence of
one-hot matmuls: edges sorted by destination window are gathered 128 at a
time with dma_gather (bf16 rows), a one-hot selection matrix S is built on
the vector engine from destination offsets, and S.T @ G accumulates into a
PSUM tile holding the window's aggregate.  Dense layer matmuls, BatchNorm
and ReLU run in a feature-major layout (features on partitions) so BN
reductions are free-axis reductions and the BN+ReLU apply is a single
scalar-engine activation per window.

Throughput notes:
 - the dma_gather SWDGE ucode runs on one Q7 core pair per queue; issuing
   a block's gathers back-to-back on queues 0-3 engages all 8 Q7 cores.
 - the one-hot S tiles depend only on edge structure, which is shared by
   both layers: layer 0 builds them (one batched tile per gather) and
   stores to HBM; layer 1 reloads them with a single dense DMA per gather
   instead of rebuilding on the vector engine.

Self-contained: only numpy + the concourse stack from /opt/trn_rl_repo.
"""
import sys

for _p in ("/opt/trn_rl_repo", "/root/.axon_site/_ro/trn_rl_repo"):
    if _p not in sys.path:
        sys.path.insert(0, _p)

import os

import numpy as np
import ml_dtypes

import concourse.bacc as bacc
import concourse.mybir as mybir
import concourse.tile as tile
from concourse.bass_utils import run_bass_kernel_spmd

F32 = mybir.dt.float32
BF16 = mybir.dt.bfloat16
I16 = mybir.dt.int16
AX = mybir.AxisListType
OP = mybir.AluOpType
AF = mybir.ActivationFunctionType

EPS = 1e-5


# --------------------------------------------------------------------------
# host-side graph preprocessing (index data only; no float math on x)
# --------------------------------------------------------------------------
def preprocess(edge_index, n_nodes, block=8, group=25000, gmax=4096):
    """Sort edges into (window-block, src-group) runs and chunk them.

    Returns metadata driving the bass program plus the packed index arrays.
    A "window" is 128 consecutive destination nodes (one PSUM tile's rows).
    A "run" is all edges with dst in one block and src in one group, sorted
    by dst, split into gathers of <= gmax indices (multiple of 128).
    Each gather's matmul work is a list of "slots": one (chunk, window)
    pair per one-hot matrix.  dl holds one pre-biased offset column per
    slot so the S build is a single is_equal against iota.
    """
    src = np.asarray(edge_index[0], dtype=np.int64)
    dst = np.asarray(edge_index[1], dtype=np.int64)
    E = src.shape[0]
    nw = (n_nodes + 127) // 128
    n_groups = (n_nodes + group - 1) // group

    deg = np.bincount(dst, minlength=n_nodes).astype(np.float32)
    inv_deg = (1.0 / np.maximum(deg, 1.0)).astype(np.float32)
    inv_pad = np.ones(nw * 128, np.float32)
    inv_pad[:n_nodes] = inv_deg
    inv_col = inv_pad.reshape(nw, 128).T.copy()  # [128, nw]

    # order all edges by (block, group, dst) in a single argsort
    blk = dst // (128 * block)
    grp = src // group
    key = (blk * n_groups + grp) * np.int64(n_nodes) + dst
    order = np.argsort(key, kind="stable")
    s_s, s_d, s_b, s_g = src[order], dst[order], blk[order], grp[order]

    # run boundaries: change of (blk, grp)
    rk = s_b * n_groups + s_g
    bounds = np.flatnonzero(np.diff(rk)) + 1
    starts = np.concatenate([[0], bounds])
    ends = np.concatenate([bounds, [E]])

    gathers = []          # per gather: dict(g, idx_off, nidx, slot_off, slots)
    idx_parts = []        # wrapped int16 [16, n/16] pieces (replicated later)
    dl_parts = []         # [128, nslots] f32 pieces (pre-biased per slot)
    idx_cursor = 0        # in int16 columns (16-wrapped)
    slot_cursor = 0       # in slots
    for s, e in zip(starts, ends):
        g = int(s_g[s])
        for q in range(s, e, gmax):
            qe = min(q + gmax, e)
            es = s_s[q:qe] - g * group
            ed = s_d[q:qe]
            ne = qe - q
            npad = (-ne) % 128
            idxs = np.concatenate([es, np.zeros(npad, np.int64)])
            dabs = np.concatenate([ed, np.full(npad, -1, np.int64)])
            nidx = ne + npad
            C = nidx // 128
            slots = []    # (ci, w)
            dl_cols = []
            for c in range(C):
                dc = dabs[c * 128:(c + 1) * 128]
                valid = dc >= 0
                wfirst = int(dc[valid].min()) // 128
                wlast = int(dc[valid].max()) // 128
                rel = np.where(valid, dc - wfirst * 128, -1).astype(np.float32)
                for k in range(wlast - wfirst + 1):
                    slots.append((c, wfirst + k))
                    dl_cols.append(rel - 128.0 * k)
            idx_parts.append(idxs.reshape(-1, 16).T.astype(np.int16))
            dl_parts.append(np.stack(dl_cols, axis=1))
            gathers.append(dict(g=g, idx_off=idx_cursor, nidx=nidx,
                                slot_off=slot_cursor, slots=slots))
            idx_cursor += nidx // 16
            slot_cursor += len(slots)

    idx_arr = np.tile(np.concatenate(idx_parts, axis=1), (8, 1))  # [128, TI]
    dl_arr = np.concatenate(dl_parts, axis=1).astype(np.float32)

    return dict(gathers=gathers, idx_arr=idx_arr, dl_arr=dl_arr,
                inv_col=inv_col, nw=nw, n_groups=n_groups, block=block,
                group=group, n_slots=slot_cursor,
                max_slots=max(len(g["slots"]) for g in gathers))


# --------------------------------------------------------------------------
# bass program
# --------------------------------------------------------------------------
def build_program(meta, n_nodes, d=128):
    nw = meta["nw"]
    block = meta["block"]
    group = meta["group"]
    gathers = meta["gathers"]
    TI = meta["idx_arr"].shape[1]
    TS = meta["n_slots"]

    ms = meta["max_slots"]

    def wsz(w):
        return min(128, n_nodes - w * 128)

    n_queues = int(os.environ.get("KERNEL_QUEUES", "4"))
    nc = bacc.Bacc("TRN2", target_bir_lowering=False, debug=False,
                   num_swdge_queues=n_queues)
    nc._kq = n_queues
    x_t = nc.declare_dram_parameter("x", [n_nodes, d], F32, isOutput=False)
    idx_t = nc.declare_dram_parameter("idx", [128, TI], I16, isOutput=False)
    dl_t = nc.declare_dram_parameter("dl", [128, TS], F32, isOutput=False)
    invd_t = nc.declare_dram_parameter("invd", [128, nw], F32, isOutput=False)
    iota_t = nc.declare_dram_parameter("iota", [128, 128], F32, isOutput=False)
    ident_t = nc.declare_dram_parameter("ident", [128, 128], F32, isOutput=False)
    identb_t = nc.declare_dram_parameter("identb", [128, 128], BF16, isOutput=False)
    ws_t = nc.declare_dram_parameter("Wself", [2, d, d], F32, isOutput=False)
    wn_t = nc.declare_dram_parameter("Wnbr", [2, d, d], F32, isOutput=False)
    wsb_t = nc.declare_dram_parameter("Wselfb", [d, d], BF16, isOutput=False)  # layer-1 self, bf16
    b_t = nc.declare_dram_parameter("bias", [2, d, 1], F32, isOutput=False)
    gam_t = nc.declare_dram_parameter("gamma", [d, 1], F32, isOutput=False)
    bet_t = nc.declare_dram_parameter("beta", [d, 1], F32, isOutput=False)
    out_t = nc.declare_dram_parameter("out", [n_nodes, d], F32, isOutput=True)

    xb_t = nc.dram_tensor("xb", [n_nodes, d], BF16)          # bf16 copy of x
    h1s_t = nc.dram_tensor("h1s", [nw, d, 128], F32)          # h1 pre-act, feat-major
    h1a_t = nc.dram_tensor("h1a", [n_nodes, d], BF16)         # h1 post BN+relu, node-major
    # cached one-hot tiles, split into two tensors to stay under the 256MB
    # DRAM scratch page size; each gather's slot run lives in one tensor.
    half = TS // 2
    split_gi = next((i for i, g in enumerate(gathers)
                     if g["slot_off"] + len(g["slots"]) > half), len(gathers))
    split_slot = (gathers[split_gi]["slot_off"] if split_gi < len(gathers)
                  else TS)
    s_t0 = nc.dram_tensor("sone0", [128, max(split_slot, 1) * 128], BF16)
    s_t1 = nc.dram_tensor("sone1", [128, max(TS - split_slot, 1) * 128], BF16)

    def s_slice(slot_off, ns):
        if slot_off >= split_slot:
            off = slot_off - split_slot
            return s_t1[:, off * 128:(off + ns) * 128]
        return s_t0[:, slot_off * 128:(slot_off + ns) * 128]

    n_blocks = (nw + block - 1) // block

    # hoisted num_idxs registers (one per distinct gather length)
    nidx_regs = {}

    def nidx_reg(n):
        if n not in nidx_regs:
            nidx_regs[n] = nc.gpsimd.to_reg(n)
        return nidx_regs[n]

    # ---------------- build ----------------
    with tile.TileContext(nc) as tc:
        # ---- phase 0: bf16 copy of x (HBM->HBM cast via SBUF tiles) ----
        with tc.tile_pool(name="cast", bufs=3) as castp:
            nrb = (n_nodes + 511) // 512
            for i in range(nrb):
                r0 = i * 512
                rows = min(512, n_nodes - r0)
                ft = castp.tile([128, 4, d], F32, tag="cf", name=f"cf{i}")
                bt = castp.tile([128, 4, d], BF16, tag="cb", name=f"cb{i}")
                if rows == 512:
                    src = x_t[r0:r0 + 512, :].rearrange("(a p) d -> p a d", p=128)
                    dstv = xb_t[r0:r0 + 512, :].rearrange("(a p) d -> p a d", p=128)
                    nc.sync.dma_start(ft[:], src)
                    nc.vector.tensor_copy(bt[:], ft[:])
                    nc.sync.dma_start(dstv, bt[:])
                else:
                    done = 0
                    while done < rows:
                        pr = min(128, rows - done)
                        srcv = x_t[r0 + done:r0 + done + pr, :]
                        dstv = xb_t[r0 + done:r0 + done + pr, :]
                        nc.sync.dma_start(ft[:pr, 0, :], srcv)
                        nc.vector.tensor_copy(bt[:pr, 0, :], ft[:pr, 0, :])
                        nc.sync.dma_start(dstv, bt[:pr, 0, :])
                        done += pr

        # ---- constants ----
        with tc.tile_pool(name="const", bufs=1) as constpool:
            iota_sb = constpool.tile([128, 128], F32)
            ident_sb = constpool.tile([128, 128], F32)
            identb_sb = constpool.tile([128, 128], BF16)
            invd_sb = constpool.tile([128, nw], F32)
            ws0_sb = constpool.tile([128, 128], F32)
            wn0_sb = constpool.tile([128, 128], F32)
            wn1_sb = constpool.tile([128, 128], F32)
            wsb_sb = constpool.tile([128, 128], BF16)
            b0_sb = constpool.tile([128, 1], F32)
            b1_sb = constpool.tile([128, 1], F32)
            gam_sb = constpool.tile([128, 1], F32)
            bet_sb = constpool.tile([128, 1], F32)
            stats_sum = constpool.tile([128, nw], F32)
            stats_sq = constpool.tile([128, nw], F32)
            a_sb = constpool.tile([128, 1], F32)
            c_sb = constpool.tile([128, 1], F32)
            nc.sync.dma_start(iota_sb[:], iota_t[:])
            nc.sync.dma_start(ident_sb[:], ident_t[:])
            nc.sync.dma_start(identb_sb[:], identb_t[:])
            nc.sync.dma_start(invd_sb[:], invd_t[:])
            nc.sync.dma_start(ws0_sb[:], ws_t[0])
            nc.sync.dma_start(wn0_sb[:], wn_t[0])
            nc.sync.dma_start(wn1_sb[:], wn_t[1])
            nc.sync.dma_start(wsb_sb[:], wsb_t[:])
            nc.sync.dma_start(b0_sb[:], b_t[0])
            nc.sync.dma_start(b1_sb[:], b_t[1])
            nc.sync.dma_start(gam_sb[:], gam_t[:])
            nc.sync.dma_start(bet_sb[:], bet_t[:])

            # ---- phase A: agg0 + layer0 + BN stats ----
            with (
                tc.tile_pool(name="gath", bufs=8) as gathp,
                tc.tile_pool(name="idxp", bufs=8) as idxp,
                tc.tile_pool(name="dlp", bufs=8) as dlp,
                tc.tile_pool(name="sp", bufs=4) as sp,
                tc.tile_pool(name="aggp", bufs=3) as aggp,
                tc.tile_pool(name="xp", bufs=3) as xp,
                tc.tile_pool(name="h1p", bufs=3) as h1p,
                tc.tile_pool(name="wpsp", bufs=1, space="PSUM") as wpsp,
                tc.tile_pool(name="pstp", bufs=3, space="PSUM") as pstp,
                tc.tile_pool(name="php", bufs=2, space="PSUM") as php,
            ):
                def consumeA(w, aT):
                    n = wsz(w)
                    xw = xp.tile([128, 128], F32, tag="xw", name=f"xw{w}")
                    nc.sync.dma_start(xw[:n, :], x_t[w * 128:w * 128 + n, :])
                    ptx = pstp.tile([128, 128], F32, tag="pt", name=f"ptx{w}")
                    nc.tensor.transpose(ptx[:, :n], xw[:n, :], ident_sb[:n, :n])
                    xT = xp.tile([128, 128], F32, tag="xT", name=f"xT{w}")
                    nc.vector.tensor_copy(xT[:, :n], ptx[:, :n])
                    hp = php.tile([128, 128], F32, tag="hp", name=f"hp{w}")
                    nc.tensor.matmul(hp[:, :n], ws0_sb[:], xT[:, :n],
                                     start=True, stop=False)
                    nc.tensor.matmul(hp[:, :n], wn0_sb[:], aT[:, :n],
                                     start=False, stop=True)
                    h1 = h1p.tile([128, 128], F32, tag="h1", name=f"h1_{w}")
                    nc.vector.tensor_scalar(h1[:, :n], hp[:, :n], b0_sb[:],
                                            None, OP.add, OP.add,
                                            accum_out=stats_sum[:, w:w + 1])
                    sq = h1p.tile([128, 128], F32, tag="sq", name=f"sq{w}")
                    nc.scalar.activation(sq[:, :n], h1[:, :n], AF.Square,
                                         accum_out=stats_sq[:, w:w + 1])
                    nc.sync.dma_start(h1s_t[w][:, :n], h1[:, :n])

                run_agg(nc, tc, gathers, n_blocks, block, nw, d, ms,
                        iota_sb, ident_sb, invd_sb,
                        gathp, idxp, dlp, sp, aggp, pstp, wpsp,
                        lambda g: xb_t[g * group:min((g + 1) * group, n_nodes), :],
                        idx_t, dl_t, s_slice, consumeA, "A", nidx_reg,
                        build_s=True)

            # ---- BN stat finalize ----
            with tc.tile_pool(name="bnf", bufs=1) as bnf:
                sum_tot = bnf.tile([128, 1], F32)
                sq_tot = bnf.tile([128, 1], F32)
                nc.vector.reduce_sum(sum_tot[:], stats_sum[:], AX.X)
                nc.vector.reduce_sum(sq_tot[:], stats_sq[:], AX.X)
                mean = bnf.tile([128, 1], F32)
                msq = bnf.tile([128, 1], F32)
                nc.scalar.mul(mean[:], sum_tot[:], 1.0 / n_nodes)
                nc.scalar.mul(msq[:], sq_tot[:], 1.0 / n_nodes)
                m2 = bnf.tile([128, 1], F32)
                nc.vector.tensor_scalar(m2[:], mean[:], mean[:], None, OP.mult)
                var = bnf.tile([128, 1], F32)
                nc.vector.tensor_scalar(var[:], msq[:], m2[:], None, OP.subtract)
                vare = bnf.tile([128, 1], F32)
                nc.vector.tensor_scalar(vare[:], var[:], float(EPS), None, OP.add)
                std = bnf.tile([128, 1], F32)
                nc.scalar.activation(std[:], vare[:], AF.Sqrt, bias=0.0)
                rstd = bnf.tile([128, 1], F32)
                nc.vector.reciprocal(rstd[:], std[:])
                nc.vector.tensor_scalar(a_sb[:], gam_sb[:], rstd[:], None, OP.mult)
                ma = bnf.tile([128, 1], F32)
                nc.vector.tensor_scalar(ma[:], mean[:], a_sb[:], None, OP.mult)
                nc.vector.tensor_scalar(c_sb[:], bet_sb[:], ma[:], None, OP.subtract)

            # ---- phase B: BN apply + relu -> h1a (bf16 node-major) ----
            with (
                tc.tile_pool(name="pb", bufs=4) as pb,
                tc.tile_pool(name="pbps", bufs=2, space="PSUM") as pbps,
            ):
                for w in range(nw):
                    n = wsz(w)
                    ht = pb.tile([128, 128], F32, tag="ht", name=f"bht{w}")
                    nc.sync.dma_start(ht[:, :n], h1s_t[w][:, :n])
                    ab = pb.tile([128, 128], BF16, tag="ab", name=f"bab{w}")
                    nc.scalar.activation(ab[:, :n], ht[:, :n], AF.Relu,
                                         bias=c_sb[:], scale=a_sb[:])
                    pt = pbps.tile([128, 128], BF16, tag="bpt", name=f"bpt{w}")
                    nc.tensor.transpose(pt[:n, :], ab[:, :n], identb_sb[:])
                    hn = pb.tile([128, 128], BF16, tag="hn", name=f"bhn{w}")
                    nc.vector.tensor_copy(hn[:n, :], pt[:n, :])
                    nc.sync.dma_start(h1a_t[w * 128:w * 128 + n, :], hn[:n, :])

            # ---- phase C: agg1 + layer1 -> out ----
            with (
                tc.tile_pool(name="gathC", bufs=8) as gathp,
                tc.tile_pool(name="idxpC", bufs=8) as idxp,
                tc.tile_pool(name="dlpC", bufs=8) as dlp,
                tc.tile_pool(name="spC", bufs=4) as sp,
                tc.tile_pool(name="aggpC", bufs=3) as aggp,
                tc.tile_pool(name="xpC", bufs=3) as xp,
                tc.tile_pool(name="h2p", bufs=3) as h2p,
                tc.tile_pool(name="wpspC", bufs=1, space="PSUM") as wpsp,
                tc.tile_pool(name="pstpC", bufs=3, space="PSUM") as pstp,
                tc.tile_pool(name="phpC", bufs=2, space="PSUM") as php,
            ):
                def consumeC(w, aT):
                    n = wsz(w)
                    hw = xp.tile([128, 128], BF16, tag="hw", name=f"chw{w}")
                    nc.sync.dma_start(hw[:n, :], h1a_t[w * 128:w * 128 + n, :])
                    pth = pstp.tile([128, 128], BF16, tag="pt", name=f"cpt{w}")
                    nc.tensor.transpose(pth[:, :n], hw[:n, :], identb_sb[:n, :n])
                    hT = xp.tile([128, 128], BF16, tag="hT", name=f"chT{w}")
                    nc.vector.tensor_copy(hT[:, :n], pth[:, :n])
                    hp = php.tile([128, 128], F32, tag="hp2", name=f"chp{w}")
                    nc.tensor.matmul(hp[:, :n], wsb_sb[:], hT[:, :n],
                                     start=True, stop=False)
                    nc.tensor.matmul(hp[:, :n], wn1_sb[:], aT[:, :n],
                                     start=False, stop=True)
                    h2T = h2p.tile([128, 128], F32, tag="h2T", name=f"ch2T{w}")
                    nc.vector.tensor_scalar(h2T[:, :n], hp[:, :n], b1_sb[:],
                                            None, OP.add)
                    pto = pstp.tile([128, 128], F32, tag="pt", name=f"cpto{w}")
                    nc.tensor.transpose(pto[:n, :], h2T[:, :n], ident_sb[:])
                    h2n = h2p.tile([128, 128], F32, tag="h2n", name=f"ch2n{w}")
                    nc.vector.tensor_copy(h2n[:n, :], pto[:n, :])
                    nc.sync.dma_start(out_t[w * 128:w * 128 + n, :], h2n[:n, :])

                run_agg(nc, tc, gathers, n_blocks, block, nw, d, ms,
                        iota_sb, ident_sb, invd_sb,
                        gathp, idxp, dlp, sp, aggp, pstp, wpsp,
                        lambda g: h1a_t[g * group:min((g + 1) * group, n_nodes), :],
                        idx_t, dl_t, s_slice, consumeC, "C", nidx_reg,
                        build_s=False)

    nc.compile()
    return nc


def run_agg(nc, tc, gathers, n_blocks, block, nw, d, ms,
            iota_sb, ident_sb, invd_sb, gathp, idxp, dlp, sp, aggp, pstp,
            wpsp, src_fn, idx_t, dl_t, s_slice, consume, tag, nidx_reg,
            build_s):
    """Emit the aggregation instruction stream for one layer.

    build_s=True: build the one-hot tiles on the vector engine and store
    the whole batch to s_t.  build_s=False: reload them from s_t.
    """
    # group gathers by block (slots of one gather all lie in one block)
    by_block = [[] for _ in range(n_blocks)]
    for gi, ga in enumerate(gathers):
        bi = ga["slots"][0][1] // block
        by_block[bi].append((gi, ga))

    # per-bank (block, half) first/last matmul ids: start=True zeroes the
    # whole 2KB PSUM zero-region (= bank), so only the first matmul into a
    # bank may set start; later windows' slices zero on first touch.
    bank_first = {}
    bank_last = {}
    for bi in range(n_blocks):
        wlo = bi * block
        for gi, ga in by_block[bi]:
            for si, (ci, w) in enumerate(ga["slots"]):
                key = (bi, (w - wlo) // 4)
                bank_first.setdefault(key, (gi, si))
                bank_last[key] = (gi, si)

    seen = set()
    kq = getattr(nc, "_kq", 1)
    g_ctr = 0
    for bi in range(n_blocks):
        wlo = bi * block
        whi = min(wlo + block, nw)
        wtiles = {}

        def pslice(w):
            half = (w - wlo) // 4
            if half not in wtiles:
                wtiles[half] = wpsp.tile(
                    [128, 512], mybir.dt.float32, tag=f"wps{half}",
                    name=f"wps_{tag}_{bi}_{half}")
            off = ((w - wlo) % 4) * 128
            return wtiles[half][:, off:off + 128]

        def finish_window(w):
            aw = aggp.tile([128, 128], mybir.dt.float32, tag="agg",
                           name=f"agg_{tag}_{w}")
            nc.vector.tensor_scalar(aw[:], pslice(w), invd_sb[:, w:w + 1],
                                    None, OP.mult)
            pt = pstp.tile([128, 128], mybir.dt.float32, tag="pt",
                           name=f"pt_{tag}_{w}")
            nc.tensor.transpose(pt[:], aw[:], ident_sb[:])
            aT = aggp.tile([128, 128], mybir.dt.float32, tag="aggT",
                           name=f"aggT_{tag}_{w}")
            nc.vector.tensor_copy(aT[:], pt[:])
            consume(w, aT)

        blockg = by_block[bi]

        # 1) index loads + gathers for the whole block, rotating queues
        gtiles = {}
        stiles = {}
        for j, (gi, ga) in enumerate(blockg):
            nidx = ga["nidx"]
            C = nidx // 128
            i16c = nidx // 16
            ns = len(ga["slots"])
            idx_sb = idxp.tile([128, i16c], I16, tag="idx",
                               name=f"idx_{tag}_{gi}")
            nc.sync.dma_start(
                idx_sb[:], idx_t[:, ga["idx_off"]:ga["idx_off"] + i16c])
            if build_s:
                dl_sb = dlp.tile([128, ns], mybir.dt.float32, tag="dl",
                                 name=f"dl_{tag}_{gi}")
                nc.sync.dma_start(
                    dl_sb[:], dl_t[:, ga["slot_off"]:ga["slot_off"] + ns])
            gdst = gathp.tile([128, C, d], BF16, tag="gd",
                              name=f"gd_{tag}_{gi}")
            nc.gpsimd.dma_gather(gdst[:], src_fn(ga["g"]), idx_sb[:],
                                 nidx, nidx_reg(nidx), d, single_packet=False,
                                 queue_num=g_ctr % kq)
            g_ctr += 1
            gtiles[gi] = gdst
            Sg = sp.tile([128, ms, 128], BF16, tag="S",
                         name=f"S_{tag}_{gi}")
            stiles[gi] = Sg
            soff = ga["slot_off"]
            if build_s:
                for si in range(ns):
                    nc.vector.tensor_scalar(
                        Sg[:, si, :], iota_sb[:], dl_sb[:, si:si + 1],
                        None, OP.is_equal)
                nc.sync.dma_start(s_slice(soff, ns), Sg[:, :ns, :])
            else:
                nc.sync.dma_start(Sg[:, :ns, :], s_slice(soff, ns))

        # 2) matmuls + window finishes
        for gi, ga in blockg:
            gdst = gtiles[gi]
            Sg = stiles[gi]
            for si, (ci, w) in enumerate(ga["slots"]):
                key = (bi, (w - wlo) // 4)
                is_bank_last = bank_last[key] == (gi, si)
                nc.tensor.matmul(
                    pslice(w), Sg[:, si, :], gdst[:, ci, :],
                    start=bank_first[key] == (gi, si),
                    stop=is_bank_last)
                seen.add(w)
                if is_bank_last:
                    half = (w - wlo) // 4
                    for wv in range(wlo + half * 4,
                                    min(wlo + half * 4 + 4, whi)):
                        if wv in seen:
                            finish_window(wv)

        for w in range(wlo, whi):
            if w not in seen:
                seen.add(w)
                aT = aggp.tile([128, 128], mybir.dt.float32, tag="aggT",
                               name=f"aggzT_{tag}_{w}")
                nc.vector.memset(aT[:], 0.0)
                consume(w, aT)


# --------------------------------------------------------------------------
# public entry point
# --------------------------------------------------------------------------
def kernel(x, edge_index, W_self, W_nbr, b, gamma, beta):
    x = np.asarray(x, dtype=np.float32)
    edge_index = np.asarray(edge_index)
    W_self = np.asarray(W_self, dtype=np.float32)
    W_nbr = np.asarray(W_nbr, dtype=np.float32)
    b = np.asarray(b, dtype=np.float32)
    gamma = np.asarray(gamma, dtype=np.float32)
    beta = np.asarray(beta, dtype=np.float32)

    n_nodes, d = x.shape
    n_experts = W_self.shape[0]

    meta = preprocess(edge_index, n_nodes)
    nc = build_program(meta, n_nodes, d)

    iota_np = np.tile(np.arange(128, dtype=np.float32)[None, :], (128, 1))
    in_common = {
        "x": x,
        "idx": meta["idx_arr"],
        "dl": np.asarray(meta["dl_arr"]),
        "invd": meta["inv_col"],
        "iota": iota_np,
        "ident": np.eye(128, dtype=np.float32),
        "identb": np.eye(128, dtype=ml_dtypes.bfloat16),
    }
    in_maps = []
    for e in range(n_experts):
        m = dict(in_common)
        m["Wself"] = W_self[e]
        m["Wnbr"] = W_nbr[e]
        m["Wselfb"] = W_self[e, 1].astype(ml_dtypes.bfloat16)
        m["bias"] = b[e][:, :, None]
        m["gamma"] = gamma[e, 0][:, None]
        m["beta"] = beta[e, 0][:, None]
        in_maps.append(m)

    res = run_bass_kernel_spmd(nc, in_maps, list(range(n_experts)))
    outs = [np.asarray(res.results[e]["out"]) for e in range(n_experts)]
    return np.stack(outs, axis=-1)



# revision 5
# speedup vs baseline: 1.2049x; 1.2049x over previous
"""Trainium2 Bass kernel for nn_EuclideanExperts (8-expert 2-layer GraphSAGE).

v2: destination-sharded data parallel over 8 NeuronCores, all experts on
every core.  Each core owns 1/8 of the destination nodes and only that
shard's edges (~200K instead of 1.6M), cutting SWDGE gather-descriptor
work 8x vs the expert-parallel baseline:

 - layer 0: gather x[src] rows (f32, 512B) for the shard's edges, one-hot
   aggregate per dst window, then the dense self/nbr matmuls for all 8
   experts.  BN statistics accumulate pre-bias so padded rows contribute
   exactly zero; they are AllReduced (tiny) across cores.
 - layer 1: each core needs h1 activations of arbitrary src nodes for all
   experts, so post-BN features are stored node-major as 2KB rows
   [node, 8*128] bf16, AllGathered (26MB -> 206MB), and gathered per edge
   with one 2KB descriptor covering all 8 experts at once.

The edge->window one-hot matmul machinery follows the baseline: edges are
sorted by (block of 2 windows, src group, dst); each 128-edge chunk gets a
one-hot S built on the vector engine and S.T @ G accumulates into the
window's PSUM tile.  Runs are padded to the max count over the 8 cores so
the single SPMD program is identical everywhere; pads gather row 0 of the
group (valid) with dst label -1 (matches nothing).

Self-contained: only numpy + the concourse stack from /opt/trn_rl_repo.
"""
import sys

for _p in ("/opt/trn_rl_repo", "/root/.axon_site/_ro/trn_rl_repo"):
    if _p not in sys.path:
        sys.path.insert(0, _p)

import os

import numpy as np
import ml_dtypes

import concourse.bacc as bacc
import concourse.mybir as mybir
import concourse.tile as tile
from concourse.bass_utils import run_bass_kernel_spmd

F32 = mybir.dt.float32
BF16 = mybir.dt.bfloat16
I16 = mybir.dt.int16
AX = mybir.AxisListType
OP = mybir.AluOpType
AF = mybir.ActivationFunctionType

EPS = 1e-5
N_CORES = 8
N_EXPERTS = 8
D = 128


# --------------------------------------------------------------------------
# host-side graph preprocessing (index data only; no float math on x)
# --------------------------------------------------------------------------
def preprocess(edge_index, n_nodes, n_cores=N_CORES, wpb=2, group_rows=25088):
    """Shard edges by destination, sort into (block, group, dst) runs.

    Returns per-core packed int16 gather indices and f32 one-hot labels,
    plus the global (core-uniform) run caps that shape the program.
    A block is `wpb` windows (wpb*128 dst nodes); a group is `group_rows`
    rows of the gather table (int16 index range).  Every (block, group)
    run is padded to cap[b,g] = ceil128(max over cores) so the single
    SPMD program matches all cores; pad indices gather row 0 of the
    group, pad labels are -1 (one-hot matches nothing).
    """
    src = np.asarray(edge_index[0]).astype(np.int64)
    dst = np.asarray(edge_index[1]).astype(np.int64)
    nw_tot = (n_nodes + 127) // 128
    nw = (nw_tot + n_cores - 1) // n_cores          # windows per core
    if nw % wpb:
        nw += wpb - nw % wpb                        # whole blocks
    shard = nw * 128                                # nodes per core
    ntot = shard * n_cores                          # padded node count
    nb = nw // wpb                                  # blocks per core
    ng = (ntot + group_rows - 1) // group_rows      # src groups
    assert group_rows <= 32767 + 1

    deg = np.bincount(dst, minlength=n_nodes).astype(np.float32)
    inv_deg = (1.0 / np.maximum(deg, 1.0)).astype(np.float32)
    inv_pad = np.ones(ntot, np.float32)
    inv_pad[:n_nodes] = inv_deg
    # invd per core: [128, nw] window-major columns
    invd_cores = [
        inv_pad[c * shard:(c + 1) * shard].reshape(nw, 128).T.copy()
        for c in range(n_cores)
    ]

    core_of = dst // shard
    rel = dst - core_of * shard
    blk = rel // (wpb * 128)
    grp = src // group_rows

    # per-core sorted edge arrays
    per_core = []
    for c in range(n_cores):
        m = core_of == c
        s, r, b, g = src[m], rel[m], blk[m], grp[m]
        key = (b * ng + g) * (1 << 18) + r
        o = np.argsort(key, kind="stable")
        per_core.append((s[o], r[o], b[o], g[o]))

    # run counts and caps
    counts = np.zeros((n_cores, nb, ng), np.int64)
    for c in range(n_cores):
        _, _, b, g = per_core[c]
        np.add.at(counts[c], (b, g), 1)
    caps = counts.max(axis=0)
    caps = ((caps + 127) // 128) * 128              # ceil to chunk size

    # pack idx / dl per core
    idx_cores, dl_cores = [], []
    for c in range(n_cores):
        s, r, b, g = per_core[c]
        run_id = b * ng + g
        bounds = np.searchsorted(run_id, np.arange(nb * ng + 1))
        idx_parts, dl_parts = [], []
        for bi in range(nb):
            for gi in range(ng):
                cap = int(caps[bi, gi])
                if cap == 0:
                    continue
                lo, hi = bounds[bi * ng + gi], bounds[bi * ng + gi + 1]
                es = s[lo:hi] - gi * group_rows
                rl = r[lo:hi] - bi * wpb * 128      # 0 .. wpb*128-1
                npad = cap - (hi - lo)
                es = np.concatenate([es, np.zeros(npad, np.int64)])
                rl = np.concatenate([rl, np.full(npad, -1.0)]).astype(np.float32)
                idx_parts.append(es.reshape(-1, 16).T.astype(np.int16))
                for ci in range(cap // 128):
                    col = rl[ci * 128:(ci + 1) * 128]
                    for k in range(wpb):
                        shifted = col - 128.0 * k
                        dl_parts.append(np.where(col < 0, -1.0, shifted))
        idx_cores.append(np.tile(np.concatenate(idx_parts, axis=1), (8, 1)))
        dl_cores.append(np.stack(dl_parts, axis=1).astype(np.float32))

    return dict(caps=caps, idx_cores=idx_cores, dl_cores=dl_cores,
                invd_cores=invd_cores, nw=nw, nb=nb, ng=ng, wpb=wpb,
                shard=shard, ntot=ntot, group_rows=group_rows,
                n_nodes=n_nodes)


# --------------------------------------------------------------------------
# bass program
# --------------------------------------------------------------------------
def build_program(meta, d=D, ne=N_EXPERTS, n_cores=N_CORES):
    caps = meta["caps"]
    nw, nb, ng, wpb = meta["nw"], meta["nb"], meta["ng"], meta["wpb"]
    shard, ntot = meta["shard"], meta["ntot"]
    group_rows = meta["group_rows"]
    n_nodes = meta["n_nodes"]
    TI = int(caps.sum()) // 16
    n_chunks = int(caps.sum()) // 128
    TS = n_chunks * wpb
    de = d * ne

    n_queues = int(os.environ.get("KERNEL_QUEUES", "4"))
    nc = bacc.Bacc("TRN2", target_bir_lowering=False, debug=False,
                   num_swdge_queues=n_queues)

    xp_t = nc.declare_dram_parameter("xp", [ntot, d], F32, isOutput=False)
    xsh_t = nc.declare_dram_parameter("xsh", [shard, d], F32, isOutput=False)
    idx_t = nc.declare_dram_parameter("idx", [128, TI], I16, isOutput=False)
    dl_t = nc.declare_dram_parameter("dl", [128, TS], F32, isOutput=False)
    invd_t = nc.declare_dram_parameter("invd", [128, nw], F32, isOutput=False)
    iota_t = nc.declare_dram_parameter("iota", [128, 128], F32, isOutput=False)
    ident_t = nc.declare_dram_parameter("ident", [128, 128], F32, isOutput=False)
    identb_t = nc.declare_dram_parameter("identb", [128, 128], BF16, isOutput=False)
    ws0_t = nc.declare_dram_parameter("Ws0", [ne, d, d], F32, isOutput=False)
    wn0_t = nc.declare_dram_parameter("Wn0", [ne, d, d], F32, isOutput=False)
    ws1b_t = nc.declare_dram_parameter("Ws1b", [ne, d, d], BF16, isOutput=False)
    wn1_t = nc.declare_dram_parameter("Wn1", [ne, d, d], F32, isOutput=False)
    b0_t = nc.declare_dram_parameter("b0", [ne, d, 1], F32, isOutput=False)
    b1_t = nc.declare_dram_parameter("b1", [ne, d, 1], F32, isOutput=False)
    gam_t = nc.declare_dram_parameter("gamma", [ne, d, 1], F32, isOutput=False)
    bet_t = nc.declare_dram_parameter("beta", [ne, d, 1], F32, isOutput=False)
    out_t = nc.declare_dram_parameter("out", [shard, de], F32, isOutput=True)

    h1s_t = nc.dram_tensor("h1s", [nw, ne, d, 128], BF16)     # pre-BN, feat-major
    h1afm_t = nc.dram_tensor("h1afm", [nw, ne, d, 128], BF16)  # post-BN, feat-major
    h1loc_t = nc.dram_tensor("h1loc", [shard, de], BF16)       # post-BN, node-major
    h1all_t = nc.dram_tensor("h1all", [ntot, de], BF16, addr_space="Shared")
    stats_in_t = nc.dram_tensor("stats_in", [128, 2 * ne], F32)
    stats_out_t = nc.dram_tensor("stats_out", [128, 2 * ne], F32)

    nidx_regs = {}

    def nidx_reg(n):
        if n not in nidx_regs:
            nidx_regs[n] = nc.gpsimd.to_reg(n)
        return nidx_regs[n]

    # chunk -> dl column bookkeeping must match preprocess packing order
    idx_off = np.zeros((nb, ng), np.int64)
    dl_off = np.zeros((nb, ng), np.int64)
    io, do = 0, 0
    for bi in range(nb):
        for gi in range(ng):
            idx_off[bi, gi] = io
            dl_off[bi, gi] = do
            io += int(caps[bi, gi]) // 16
            do += (int(caps[bi, gi]) // 128) * wpb

    g_ctr = [0]

    def emit_agg(bi, pools, src_t, src_dt, elem, expert_slices, psum_for,
                 bank_of, tag):
        """Gathers + one-hot matmuls for one block.  expert_slices: list of
        (e, col_lo) feature slices of the gathered rows; psum_for(k, e)
        returns the PSUM AP accumulating window (bi*wpb + k, e); bank_of
        maps (k, e) to a PSUM bank id (start=True once per bank)."""
        gathp, idxp, dlp, sp = pools
        glist = [gi for gi in range(ng) if caps[bi, gi] > 0]
        if not glist:
            return False
        gtiles = {}
        for gi in glist:
            cap = int(caps[bi, gi])
            cols = cap // 16
            it = idxp.tile([128, cols], I16, tag="idx", name=f"idx{tag}_{bi}_{gi}")
            nc.sync.dma_start(it[:], idx_t[:, idx_off[bi, gi]:idx_off[bi, gi] + cols])
            gt = gathp.tile([128, cap // 128, elem], src_dt, tag=f"gd{gi}",
                            name=f"gd{tag}_{bi}_{gi}")
            lo = gi * group_rows
            hi = min(lo + group_rows, ntot)
            nc.gpsimd.dma_gather(gt[:], src_t[lo:hi, :], it[:],
                                 cap, nidx_reg(cap), elem, single_packet=False,
                                 queue_num=g_ctr[0] % n_queues)
            g_ctr[0] += 1
            gtiles[gi] = gt
        # total chunks this block
        ctot = sum(int(caps[bi, gi]) // 128 for gi in glist)
        dcols = ctot * wpb
        dlt = dlp.tile([128, dcols], F32, tag="dl", name=f"dl{tag}_{bi}")
        d0 = int(dl_off[bi, glist[0]])
        nc.sync.dma_start(dlt[:], dl_t[:, d0:d0 + dcols])
        sdt = BF16 if src_dt == BF16 else F32
        # first (k, e) touching each bank within a chunk's matmul sequence
        seen_banks = set()
        bank_first = set()
        for k in range(wpb):
            for e, _ in expert_slices:
                bk = bank_of(k, e)
                if bk not in seen_banks:
                    seen_banks.add(bk)
                    bank_first.add((k, e))
        ci_all = 0
        for gi in glist:
            gt = gtiles[gi]
            for ci in range(int(caps[bi, gi]) // 128):
                st = sp.tile([128, wpb, 128], sdt, tag="S",
                             name=f"S{tag}_{bi}_{gi}_{ci}")
                for k in range(wpb):
                    nc.vector.tensor_scalar(
                        st[:, k, :], iota_sb[:],
                        dlt[:, ci_all * wpb + k:ci_all * wpb + k + 1],
                        None, OP.is_equal)
                for k in range(wpb):
                    for e, col_lo in expert_slices:
                        nc.tensor.matmul(
                            psum_for(k, e), st[:, k, :],
                            gt[:, ci, col_lo:col_lo + d],
                            start=ci_all == 0 and (k, e) in bank_first,
                            stop=ci_all == ctot - 1)
                ci_all += 1
        return True

    with tile.TileContext(nc) as tc:
        with tc.tile_pool(name="const", bufs=1) as constp:
            iota_sb = constp.tile([128, 128], F32)
            ident_sb = constp.tile([128, 128], F32)
            identb_sb = constp.tile([128, 128], BF16)
            invd_sb = constp.tile([128, nw], F32)
            nc.sync.dma_start(iota_sb[:], iota_t[:])
            nc.sync.dma_start(ident_sb[:], ident_t[:])
            nc.sync.dma_start(identb_sb[:], identb_t[:])
            nc.sync.dma_start(invd_sb[:], invd_t[:])
            ws0_sb, wn0_sb, ws1_sb, wn1_sb = [], [], [], []
            b0_sb, b1_sb, gam_sb, bet_sb = [], [], [], []
            for e in range(ne):
                t = constp.tile([128, 128], F32, name=f"ws0_{e}")
                nc.sync.dma_start(t[:], ws0_t[e])
                ws0_sb.append(t)
                t = constp.tile([128, 128], F32, name=f"wn0_{e}")
                nc.sync.dma_start(t[:], wn0_t[e])
                wn0_sb.append(t)
                t = constp.tile([128, 128], BF16, name=f"ws1_{e}")
                nc.sync.dma_start(t[:], ws1b_t[e])
                ws1_sb.append(t)
                t = constp.tile([128, 128], F32, name=f"wn1_{e}")
                nc.sync.dma_start(t[:], wn1_t[e])
                wn1_sb.append(t)
                for lst, src in ((b0_sb, b0_t), (b1_sb, b1_t),
                                 (gam_sb, gam_t), (bet_sb, bet_t)):
                    t = constp.tile([128, 1], F32, name=f"c{len(lst)}_{id(src)}")
                    nc.sync.dma_start(t[:], src[e])
                    lst.append(t)
            stats_sum = constp.tile([128, ne, nw], F32)
            stats_sq = constp.tile([128, ne, nw], F32)
            a_sb = constp.tile([128, ne], F32)
            c_sb = constp.tile([128, ne], F32)

            # ---- phase A: L0 gather-aggregate + dense + BN stats ----
            with (
                tc.tile_pool(name="gathA", bufs=2) as gathp,
                tc.tile_pool(name="idxA", bufs=8) as idxp,
                tc.tile_pool(name="dlA", bufs=2) as dlp,
                tc.tile_pool(name="sA", bufs=4) as sp,
                tc.tile_pool(name="aggA", bufs=4) as aggp,
                tc.tile_pool(name="xA", bufs=4) as xp,
                tc.tile_pool(name="h1A", bufs=6) as h1p,
                tc.tile_pool(name="wpsA", bufs=2, space="PSUM") as wpsp,
                tc.tile_pool(name="ptA", bufs=3, space="PSUM") as pstp,
                tc.tile_pool(name="phA", bufs=3, space="PSUM") as php,
            ):
                for bi in range(nb):
                    wt = wpsp.tile([128, 512], F32, tag="wps",
                                   name=f"wpsA_{bi}")

                    def psA(k, e, wt=wt):
                        return wt[:, k * 128:(k + 1) * 128]

                    nonzero = emit_agg(bi, (gathp, idxp, dlp, sp),
                                       xp_t, F32, d, [(0, 0)], psA,
                                       lambda k, e: 0, "A")
                    for k in range(wpb):
                        w = bi * wpb + k
                        if nonzero:
                            aw = aggp.tile([128, 128], F32, tag="agg",
                                           name=f"aggA_{w}")
                            nc.vector.tensor_scalar(aw[:], psA(k, 0),
                                                    invd_sb[:, w:w + 1],
                                                    None, OP.mult)
                            pt = pstp.tile([128, 128], F32, tag="pt",
                                           name=f"ptA_{w}")
                            nc.tensor.transpose(pt[:], aw[:], ident_sb[:])
                            aT = aggp.tile([128, 128], F32, tag="aggT",
                                           name=f"aTA_{w}")
                            nc.vector.tensor_copy(aT[:], pt[:])
                        else:
                            aT = aggp.tile([128, 128], F32, tag="aggT",
                                           name=f"aTA_{w}")
                            nc.vector.memset(aT[:], 0.0)
                        xw = xp.tile([128, 128], F32, tag="xw", name=f"xwA_{w}")
                        nc.sync.dma_start(xw[:], xsh_t[w * 128:(w + 1) * 128, :])
                        ptx = pstp.tile([128, 128], F32, tag="pt",
                                        name=f"ptxA_{w}")
                        nc.tensor.transpose(ptx[:], xw[:], ident_sb[:])
                        xT = xp.tile([128, 128], F32, tag="xT", name=f"xTA_{w}")
                        nc.vector.tensor_copy(xT[:], ptx[:])
                        for e in range(ne):
                            hp = php.tile([128, 128], F32, tag="hp",
                                          name=f"hpA_{w}_{e}")
                            nc.tensor.matmul(hp[:], ws0_sb[e][:], xT[:],
                                             start=True, stop=False)
                            nc.tensor.matmul(hp[:], wn0_sb[e][:], aT[:],
                                             start=False, stop=True)
                            sq = h1p.tile([128, 128], F32, tag="sq",
                                          name=f"sqA_{w}_{e}")
                            nc.scalar.activation(
                                sq[:], hp[:], AF.Square,
                                accum_out=stats_sq[:, e, w:w + 1])
                            h1 = h1p.tile([128, 128], BF16, tag="h1",
                                          name=f"h1A_{w}_{e}")
                            nc.vector.tensor_scalar(
                                h1[:], hp[:], b0_sb[e][:], None, OP.add,
                                OP.add, accum_out=stats_sum[:, e, w:w + 1])
                            nc.sync.dma_start(h1s_t[w, e], h1[:])

            # ---- BN stats: reduce, AllReduce, finalize scale/bias ----
            with tc.tile_pool(name="bnf", bufs=1) as bnf:
                packed = bnf.tile([128, 2 * ne], F32)
                for e in range(ne):
                    nc.vector.reduce_sum(packed[:, e:e + 1],
                                         stats_sum[:, e, :], AX.X)
                    nc.vector.reduce_sum(packed[:, ne + e:ne + e + 1],
                                         stats_sq[:, e, :], AX.X)
                nc.sync.dma_start(stats_in_t[:], packed[:])
                nc.gpsimd.collective_compute(
                    "AllReduce", OP.add,
                    replica_groups=[list(range(n_cores))],
                    ins=[stats_in_t[:]], outs=[stats_out_t[:]])
                tot = bnf.tile([128, 2 * ne], F32)
                nc.sync.dma_start(tot[:], stats_out_t[:])
                # per expert: a = gamma * rsqrt(var+eps), c = beta - mean*a
                inv_n = 1.0 / float(n_nodes)
                for e in range(ne):
                    mean_pre = bnf.tile([128, 1], F32, name=f"mp{e}")
                    nc.scalar.mul(mean_pre[:], tot[:, e:e + 1], inv_n)
                    # note stats_sum accumulated post-bias: every one of the
                    # ntot rows (pads included, hp=0 there) adds exactly b0.
                    corr = float(meta["ntot"]) * inv_n
                    mpre2 = bnf.tile([128, 1], F32, name=f"mp2_{e}")
                    nc.vector.tensor_scalar(mpre2[:], b0_sb[e][:], -corr,
                                            None, OP.mult)
                    nc.vector.tensor_scalar(mean_pre[:], mean_pre[:],
                                            mpre2[:], None, OP.add)
                    msq = bnf.tile([128, 1], F32, name=f"msq{e}")
                    nc.scalar.mul(msq[:], tot[:, ne + e:ne + e + 1], inv_n)
                    m2 = bnf.tile([128, 1], F32, name=f"m2_{e}")
                    nc.vector.tensor_scalar(m2[:], mean_pre[:], mean_pre[:],
                                            None, OP.mult)
                    var = bnf.tile([128, 1], F32, name=f"var{e}")
                    nc.vector.tensor_scalar(var[:], msq[:], m2[:], None,
                                            OP.subtract)
                    vare = bnf.tile([128, 1], F32, name=f"vare{e}")
                    nc.vector.tensor_scalar(vare[:], var[:], float(EPS),
                                            None, OP.add)
                    std = bnf.tile([128, 1], F32, name=f"std{e}")
                    nc.scalar.activation(std[:], vare[:], AF.Sqrt, bias=0.0)
                    rstd = bnf.tile([128, 1], F32, name=f"rstd{e}")
                    nc.vector.reciprocal(rstd[:], std[:])
                    nc.vector.tensor_scalar(a_sb[:, e:e + 1], gam_sb[e][:],
                                            rstd[:], None, OP.mult)
                    mean_post = bnf.tile([128, 1], F32, name=f"mpost{e}")
                    nc.vector.tensor_scalar(mean_post[:], mean_pre[:],
                                            b0_sb[e][:], None, OP.add)
                    ma = bnf.tile([128, 1], F32, name=f"ma{e}")
                    nc.vector.tensor_scalar(ma[:], mean_post[:],
                                            a_sb[:, e:e + 1], None, OP.mult)
                    nc.vector.tensor_scalar(c_sb[:, e:e + 1], bet_sb[e][:],
                                            ma[:], None, OP.subtract)

            # ---- phase B: BN apply + relu; node-major 2KB rows ----
            with (
                tc.tile_pool(name="pb", bufs=4) as pb,
                tc.tile_pool(name="stg", bufs=3) as stgp,
                tc.tile_pool(name="pbps", bufs=3, space="PSUM") as pbps,
            ):
                for w in range(nw):
                    stage = stgp.tile([128, de], BF16, tag="stage",
                                      name=f"stg_{w}")
                    for e in range(ne):
                        ht = pb.tile([128, 128], BF16, tag="ht",
                                     name=f"bht{w}_{e}")
                        nc.sync.dma_start(ht[:], h1s_t[w, e])
                        ab = pb.tile([128, 128], BF16, tag="ab",
                                     name=f"bab{w}_{e}")
                        nc.scalar.activation(ab[:], ht[:], AF.Relu,
                                             bias=c_sb[:, e:e + 1],
                                             scale=a_sb[:, e:e + 1])
                        nc.sync.dma_start(h1afm_t[w, e], ab[:])
                        pt = pbps.tile([128, 128], BF16, tag="bpt",
                                       name=f"bpt{w}_{e}")
                        nc.tensor.transpose(pt[:], ab[:], identb_sb[:])
                        nc.vector.tensor_copy(stage[:, e * 128:(e + 1) * 128],
                                              pt[:])
                    nc.sync.dma_start(
                        h1loc_t[w * 128:(w + 1) * 128, :], stage[:])

            # ---- AllGather h1 across cores ----
            with tc.tile_pool(name="agp", bufs=1):
                nc.gpsimd.collective_compute(
                    "AllGather", OP.bypass,
                    replica_groups=[list(range(n_cores))],
                    ins=[h1loc_t[:]], outs=[h1all_t[:]])

            # ---- phase C: L1 gather-aggregate + dense -> out ----
            with (
                tc.tile_pool(name="gathC", bufs=1) as gathp,
                tc.tile_pool(name="idxC", bufs=8) as idxp,
                tc.tile_pool(name="dlC", bufs=2) as dlp,
                tc.tile_pool(name="sC", bufs=4) as sp,
                tc.tile_pool(name="aggC", bufs=4) as aggp,
                tc.tile_pool(name="hC", bufs=4) as hcp,
                tc.tile_pool(name="h2C", bufs=6) as h2p,
                tc.tile_pool(name="wpsC", bufs=1, space="PSUM") as wpsp,
                tc.tile_pool(name="ptC", bufs=2, space="PSUM") as pstp,
                tc.tile_pool(name="phC", bufs=2, space="PSUM") as php,
            ):
                for bi in range(nb):
                    # wpb*ne agg tiles packed into wpb*ne/4 psum banks
                    wts = [wpsp.tile([128, 512], F32, tag=f"wq{q}",
                                     name=f"wpsC_{bi}_{q}")
                           for q in range(wpb * ne // 4)]

                    def psC(k, e, wts=wts):
                        q = k * (ne // 4) + e // 4
                        off = (e % 4) * 128
                        return wts[q][:, off:off + 128]

                    nonzero = emit_agg(bi, (gathp, idxp, dlp, sp),
                                       h1all_t, BF16, de,
                                       [(e, e * d) for e in range(ne)],
                                       psC, lambda k, e: k * 2 + e // 4, "C")
                    for k in range(wpb):
                        w = bi * wpb + k
                        for e in range(ne):
                            if nonzero:
                                aw = aggp.tile([128, 128], F32, tag="agg",
                                               name=f"aggC_{w}_{e}")
                                nc.vector.tensor_scalar(aw[:], psC(k, e),
                                                        invd_sb[:, w:w + 1],
                                                        None, OP.mult)
                                pt = pstp.tile([128, 128], F32, tag="pt",
                                               name=f"ptC_{w}_{e}")
                                nc.tensor.transpose(pt[:], aw[:], ident_sb[:])
                                aT = aggp.tile([128, 128], F32, tag="aggT",
                                               name=f"aTC_{w}_{e}")
                                nc.vector.tensor_copy(aT[:], pt[:])
                            else:
                                aT = aggp.tile([128, 128], F32, tag="aggT",
                                               name=f"aTC_{w}_{e}")
                                nc.vector.memset(aT[:], 0.0)
                            hfm = hcp.tile([128, 128], BF16, tag="hfm",
                                           name=f"hfmC_{w}_{e}")
                            nc.sync.dma_start(hfm[:], h1afm_t[w, e])
                            hp2 = php.tile([128, 128], F32, tag="hp2",
                                           name=f"hp2C_{w}_{e}")
                            nc.tensor.matmul(hp2[:], ws1_sb[e][:], hfm[:],
                                             start=True, stop=False)
                            nc.tensor.matmul(hp2[:], wn1_sb[e][:], aT[:],
                                             start=False, stop=True)
                            h2 = h2p.tile([128, 128], F32, tag="h2",
                                          name=f"h2C_{w}_{e}")
                            nc.vector.tensor_scalar(h2[:], hp2[:],
                                                    b1_sb[e][:], None, OP.add)
                            pto = pstp.tile([128, 128], F32, tag="pt",
                                            name=f"ptoC_{w}_{e}")
                            nc.tensor.transpose(pto[:], h2[:], ident_sb[:])
                            h2n = h2p.tile([128, 128], F32, tag="h2n",
                                           name=f"h2nC_{w}_{e}")
                            nc.vector.tensor_copy(h2n[:], pto[:])
                            nc.sync.dma_start(
                                out_t[w * 128:(w + 1) * 128,
                                      e * 128:(e + 1) * 128], h2n[:])

    nc.compile()
    return nc


# --------------------------------------------------------------------------
# input map assembly + public entry point
# --------------------------------------------------------------------------
def make_in_maps(meta, x, W_self, W_nbr, b, gamma, beta, n_cores=N_CORES):
    n_nodes, d = x.shape
    ntot, shard = meta["ntot"], meta["shard"]
    xp = np.zeros((ntot, d), np.float32)
    xp[:n_nodes] = x
    iota_np = np.tile(np.arange(128, dtype=np.float32)[None, :], (128, 1))
    common = {
        "xp": xp,
        "iota": iota_np,
        "ident": np.eye(128, dtype=np.float32),
        "identb": np.eye(128, dtype=ml_dtypes.bfloat16),
        "Ws0": W_self[:, 0].astype(np.float32),
        "Wn0": W_nbr[:, 0].astype(np.float32),
        "Ws1b": W_self[:, 1].astype(ml_dtypes.bfloat16),
        "Wn1": W_nbr[:, 1].astype(np.float32),
        "b0": b[:, 0][:, :, None].astype(np.float32),
        "b1": b[:, 1][:, :, None].astype(np.float32),
        "gamma": gamma[:, 0][:, :, None].astype(np.float32),
        "beta": beta[:, 0][:, :, None].astype(np.float32),
    }
    in_maps = []
    for c in range(n_cores):
        m = dict(common)
        m["xsh"] = xp[c * shard:(c + 1) * shard]
        m["idx"] = meta["idx_cores"][c]
        m["dl"] = meta["dl_cores"][c]
        m["invd"] = meta["invd_cores"][c]
        in_maps.append(m)
    return in_maps


def assemble_out(meta, results, n_cores=N_CORES):
    n_nodes = meta["n_nodes"]
    big = np.concatenate([np.asarray(results[c]["out"])
                          for c in range(n_cores)], axis=0)
    return np.ascontiguousarray(
        big[:n_nodes].reshape(n_nodes, N_EXPERTS, D).transpose(0, 2, 1))


def kernel(x, edge_index, W_self, W_nbr, b, gamma, beta):
    x = np.asarray(x, dtype=np.float32)
    edge_index = np.asarray(edge_index)
    W_self = np.asarray(W_self, dtype=np.float32)
    W_nbr = np.asarray(W_nbr, dtype=np.float32)
    b = np.asarray(b, dtype=np.float32)
    gamma = np.asarray(gamma, dtype=np.float32)
    beta = np.asarray(beta, dtype=np.float32)

    meta = preprocess(edge_index, x.shape[0])
    nc = build_program(meta)
    in_maps = make_in_maps(meta, x, W_self, W_nbr, b, gamma, beta)
    res = run_bass_kernel_spmd(nc, in_maps, list(range(N_CORES)))
    return assemble_out(meta, res.results)


# revision 23
# speedup vs baseline: 1.2370x; 1.0266x over previous
"""Trainium2 Bass kernel for nn_EuclideanExperts (8-expert 2-layer GraphSAGE).

v3: destination-sharded data parallel over 8 NeuronCores, all experts on
every core.  Each core owns 1/8 of the destination nodes and only that
shard's edges (~200K instead of 1.6M):

 - layer 0: gather x[src] rows (f32, 512B) for the shard's edges, aggregate
   per 128-node dst window with a weighted one-hot matmul (S[p,i] =
   (i == dst_rel[p]) * inv_deg[dst[p]], built in ONE fused vector op), then
   dense self/nbr matmuls for all 8 experts.  The aggregation matmul puts
   the gathered chunk on the stationary side so the result comes out
   feature-major - no transposes.  BN statistics accumulate pre-bias so
   padded rows contribute exactly zero; AllReduced (tiny) across cores.
 - layer 1: post-BN features are stored node-major as 2KB rows
   [node, 8*128] bf16, AllGathered (26MB -> 206MB), gathered per edge with
   one 2KB descriptor covering all 8 experts, and aggregated with
   512-wide matmuls (4 experts per instruction into one PSUM bank).

Edges are sorted by (dst window, src group, dst); runs are padded to the
max count over the 8 cores so the single SPMD program is identical
everywhere.  Valid counts are exact per core: they are loaded from an
int32 input into gpsimd registers (trailing -1 indices generate no
descriptors).  The first blocks force full-cap valid gathers so every
gather buffer byte is initialized (later stale tails are then finite).

Output is produced feature-major [window, expert, feat, node] and
re-laid-out on the host.

Self-contained: only numpy + the concourse stack from /opt/trn_rl_repo.
"""
import sys

for _p in ("/opt/trn_rl_repo", "/root/.axon_site/_ro/trn_rl_repo"):
    if _p not in sys.path:
        sys.path.insert(0, _p)

import os

import numpy as np
import ml_dtypes

import concourse.bacc as bacc
import concourse.mybir as mybir
import concourse.tile as tile
from concourse.bass_utils import run_bass_kernel_spmd

F32 = mybir.dt.float32
BF16 = mybir.dt.bfloat16
I16 = mybir.dt.int16
I32 = mybir.dt.int32
AX = mybir.AxisListType
OP = mybir.AluOpType
AF = mybir.ActivationFunctionType

EPS = 1e-5
N_CORES = 8
N_EXPERTS = 8
D = 128


# --------------------------------------------------------------------------
# host-side graph preprocessing (index data only; no float math on x)
# --------------------------------------------------------------------------
def preprocess(edge_index, n_nodes, n_cores=N_CORES, group_rows=25088):
    """Shard edges by destination, sort into (window, group, dst) runs.

    Returns per-core packed int16 gather indices, f32 one-hot labels
    (dst offset within window and inv_deg weight per edge), exact valid
    counts, and the global (core-uniform) run caps shaping the program.
    """
    src = np.asarray(edge_index[0]).astype(np.int64)
    dst = np.asarray(edge_index[1]).astype(np.int64)
    nw_tot = (n_nodes + 127) // 128
    nw = (nw_tot + n_cores - 1) // n_cores          # windows per core
    shard = nw * 128                                # nodes per core
    ntot = shard * n_cores                          # padded node count
    ng = (ntot + group_rows - 1) // group_rows      # src groups
    assert group_rows <= 32768

    deg = np.bincount(dst, minlength=n_nodes).astype(np.float32)
    inv_deg = (1.0 / np.maximum(deg, 1.0)).astype(np.float32)
    inv_pad = np.ones(ntot, np.float32)
    inv_pad[:n_nodes] = inv_deg

    core_of = dst // shard
    rel = dst - core_of * shard
    blk = rel // 128                                # window in shard
    grp = src // group_rows

    per_core = []
    for c in range(n_cores):
        m = core_of == c
        s, r, b, g = src[m], rel[m], blk[m], grp[m]
        key = (b * ng + g) * (1 << 18) + r
        o = np.argsort(key, kind="stable")
        per_core.append((s[o], r[o], b[o], g[o]))

    counts = np.zeros((n_cores, nw, ng), np.int64)
    for c in range(n_cores):
        _, _, b, g = per_core[c]
        np.add.at(counts[c], (b, g), 1)
    caps = counts.max(axis=0)
    caps = ((caps + 127) // 128) * 128              # ceil to chunk size
    # first `initb` windows initialize every gather-buffer byte (pool
    # bufs=2 per group tag): force per-group max cap, fully-valid pads
    initb = min(2, nw)
    gmax = caps.max(axis=0)
    for bi in range(initb):
        for gi in range(ng):
            if caps[bi, gi] > 0 or gmax[gi] > 0:
                caps[bi, gi] = gmax[gi]

    idx_cores, dl_cores, cnt_cores = [], [], []
    for c in range(n_cores):
        s, r, b, g = per_core[c]
        run_id = b * ng + g
        bounds = np.searchsorted(run_id, np.arange(nw * ng + 1))
        idx_parts, dl_parts = [], []
        cnts = np.zeros(nw * ng, np.int32)
        for bi in range(nw):
            for gi in range(ng):
                cap = int(caps[bi, gi])
                if cap == 0:
                    continue
                lo, hi = bounds[bi * ng + gi], bounds[bi * ng + gi + 1]
                ne_run = hi - lo
                es = s[lo:hi] - gi * group_rows
                rl = (r[lo:hi] - bi * 128).astype(np.float32)
                iv = inv_pad[s[lo:hi] * 0 + (r[lo:hi] + c * shard)]
                npad = cap - ne_run
                if bi < initb:
                    es = np.concatenate([es, np.zeros(npad, np.int64)])
                    nvalid = cap
                else:
                    nfill = max(1 - ne_run, 0)
                    es = np.concatenate(
                        [es, np.zeros(nfill, np.int64),
                         np.full(npad - nfill, -1, np.int64)])
                    nvalid = ne_run + nfill
                rl = np.concatenate([rl, np.full(npad, -1.0, np.float32)])
                iv = np.concatenate([iv, np.zeros(npad, np.float32)])
                cnts[bi * ng + gi] = nvalid
                idx_parts.append(es.reshape(-1, 16).T.astype(np.int16))
                for ci in range(cap // 128):
                    dl_parts.append(rl[ci * 128:(ci + 1) * 128])
                    dl_parts.append(iv[ci * 128:(ci + 1) * 128])
        idx_cores.append(np.tile(np.concatenate(idx_parts, axis=1), (8, 1)))
        dl_cores.append(np.stack(dl_parts, axis=1).astype(np.float32))
        cnt_cores.append(np.tile(cnts[None, :], (128, 1)))

    return dict(caps=caps, idx_cores=idx_cores, dl_cores=dl_cores,
                cnt_cores=cnt_cores, nw=nw, ng=ng, shard=shard, ntot=ntot,
                group_rows=group_rows, n_nodes=n_nodes)


# --------------------------------------------------------------------------
# bass program
# --------------------------------------------------------------------------
def build_program(meta, d=D, ne=N_EXPERTS, n_cores=N_CORES):
    caps = meta["caps"]
    nw, ng = meta["nw"], meta["ng"]
    shard, ntot = meta["shard"], meta["ntot"]
    group_rows = meta["group_rows"]
    n_nodes = meta["n_nodes"]
    TI = int(caps.sum()) // 16
    TS = (int(caps.sum()) // 128) * 2
    de = d * ne
    neq = ne // 4                                   # expert quads

    n_queues = int(os.environ.get("KERNEL_QUEUES", "4"))
    nc = bacc.Bacc("TRN2", target_bir_lowering=False, debug=False,
                   num_swdge_queues=n_queues)

    xp_t = nc.declare_dram_parameter("xp", [ntot, d], F32, isOutput=False)
    xshfm_t = nc.declare_dram_parameter("xshfm", [d, shard], F32, isOutput=False)
    cnt_t = nc.declare_dram_parameter("cnt", [128, nw * ng], I32, isOutput=False)
    idx_t = nc.declare_dram_parameter("idx", [128, TI], I16, isOutput=False)
    dl_t = nc.declare_dram_parameter("dl", [128, TS], F32, isOutput=False)
    iota_t = nc.declare_dram_parameter("iota", [128, 128], F32, isOutput=False)
    ident_t = nc.declare_dram_parameter("ident", [128, 128], F32, isOutput=False)
    identb_t = nc.declare_dram_parameter("identb", [128, 128], BF16, isOutput=False)
    ws0_t = nc.declare_dram_parameter("Ws0", [ne, d, d], F32, isOutput=False)
    wn0_t = nc.declare_dram_parameter("Wn0", [ne, d, d], F32, isOutput=False)
    ws1b_t = nc.declare_dram_parameter("Ws1b", [ne, d, d], BF16, isOutput=False)
    wn1_t = nc.declare_dram_parameter("Wn1", [ne, d, d], F32, isOutput=False)
    b0_t = nc.declare_dram_parameter("b0", [ne, d, 1], F32, isOutput=False)
    b1_t = nc.declare_dram_parameter("b1", [ne, d, 1], F32, isOutput=False)
    gam_t = nc.declare_dram_parameter("gamma", [ne, d, 1], F32, isOutput=False)
    bet_t = nc.declare_dram_parameter("beta", [ne, d, 1], F32, isOutput=False)
    # feature-major output [window, expert, feat, node]; host re-lays-out
    out_t = nc.declare_dram_parameter("out", [nw, ne, d, 128], F32, isOutput=True)

    h1s_t = nc.dram_tensor("h1s", [nw, ne, d, 128], BF16)      # pre-BN, feat-major
    h1afm_t = nc.dram_tensor("h1afm", [nw, ne, d, 128], BF16)  # post-BN, feat-major
    h1loc_t = nc.dram_tensor("h1loc", [shard, de], BF16)       # post-BN, node-major
    h1all_t = nc.dram_tensor("h1all", [ntot, de], BF16, addr_space="Shared")
    stats_in_t = nc.dram_tensor("stats_in", [128, 2 * ne], F32)
    stats_out_t = nc.dram_tensor("stats_out", [128, 2 * ne], F32)

    cnt_regs = [nc.gpsimd.alloc_register(f"cntr{i}") for i in range(16)]
    reg_ctr = [0]

    idx_off = np.zeros((nw, ng), np.int64)
    dl_off = np.zeros((nw, ng), np.int64)
    io, do = 0, 0
    for bi in range(nw):
        for gi in range(ng):
            idx_off[bi, gi] = io
            dl_off[bi, gi] = do
            io += int(caps[bi, gi]) // 16
            do += (int(caps[bi, gi]) // 128) * 2

    g_ctr = [0]

    def emit_agg(bi, pools, src_t, src_dt, slices, psum_for, bank_of,
                 swap, tag):
        """Gathers + weighted one-hot matmuls for window `bi`.  slices:
        (key, col_lo, width) feature slices (one matmul each); psum_for(key)
        returns the accumulating PSUM AP; bank_of(key) a bank id (start=True
        once per bank).  swap=True puts the gathered chunk on the stationary
        side (feature-major out, only valid for width == d)."""
        gathp, idxp, dlp, sp = pools
        glist = [gi for gi in range(ng) if caps[bi, gi] > 0]
        if not glist:
            return False
        elem = d if src_dt == F32 else de
        gtiles = {}
        for gi in glist:
            cap = int(caps[bi, gi])
            cols = cap // 16
            it = idxp.tile([128, cols], I16, tag="idx", name=f"idx{tag}_{bi}_{gi}")
            nc.sync.dma_start(it[:], idx_t[:, idx_off[bi, gi]:idx_off[bi, gi] + cols])
            gt = gathp.tile([128, cap // 128, elem], src_dt, tag=f"gd{gi}",
                            name=f"gd{tag}_{bi}_{gi}")
            lo = gi * group_rows
            hi = min(lo + group_rows, ntot)
            reg = cnt_regs[reg_ctr[0] % len(cnt_regs)]
            reg_ctr[0] += 1
            nc.gpsimd.reg_load(reg, cnt_sb[:1, bi * ng + gi:bi * ng + gi + 1])
            nc.gpsimd.dma_gather(gt[:], src_t[lo:hi, :], it[:],
                                 cap, reg, elem, single_packet=False,
                                 queue_num=g_ctr[0] % n_queues)
            g_ctr[0] += 1
            gtiles[gi] = gt
        ctot = sum(int(caps[bi, gi]) // 128 for gi in glist)
        dlt = dlp.tile([128, ctot * 2], F32, tag="dl", name=f"dl{tag}_{bi}")
        d0 = int(dl_off[bi, glist[0]])
        nc.sync.dma_start(dlt[:], dl_t[:, d0:d0 + ctot * 2])
        sdt = BF16 if src_dt == BF16 else F32
        seen, bank_first = set(), set()
        for key, _, _ in slices:
            bk = bank_of(key)
            if bk not in seen:
                seen.add(bk)
                bank_first.add(key)
        ci_all = 0
        for gi in glist:
            gt = gtiles[gi]
            for ci in range(int(caps[bi, gi]) // 128):
                st = sp.tile([128, 128], sdt, tag="S",
                             name=f"S{tag}_{bi}_{gi}_{ci}")
                # weighted one-hot: (iota == rel) * inv_deg, one fused op
                nc.vector.tensor_scalar(
                    st[:], iota_sb[:], dlt[:, 2 * ci_all:2 * ci_all + 1],
                    dlt[:, 2 * ci_all + 1:2 * ci_all + 2],
                    OP.is_equal, OP.mult)
                for key, col_lo, width in slices:
                    if swap:
                        lhs, rhs = gt[:, ci, col_lo:col_lo + width], st[:]
                    else:
                        lhs, rhs = st[:], gt[:, ci, col_lo:col_lo + width]
                    nc.tensor.matmul(
                        psum_for(key), lhs, rhs,
                        start=ci_all == 0 and key in bank_first,
                        stop=ci_all == ctot - 1)
                ci_all += 1
        return True

    with tile.TileContext(nc) as tc:
        with tc.tile_pool(name="const", bufs=1) as constp:
            iota_sb = constp.tile([128, 128], F32)
            ident_sb = constp.tile([128, 128], F32)
            identb_sb = constp.tile([128, 128], BF16)
            cnt_sb = constp.tile([128, nw * ng], I32)
            nc.sync.dma_start(cnt_sb[:], cnt_t[:])
            nc.sync.dma_start(iota_sb[:], iota_t[:])
            nc.sync.dma_start(ident_sb[:], ident_t[:])
            nc.sync.dma_start(identb_sb[:], identb_t[:])
            ws0_sb, wn0_sb, ws1_sb, wn1_sb = [], [], [], []
            b0_sb, b1_sb, gam_sb, bet_sb = [], [], [], []
            for e in range(ne):
                for lst, src, dt_ in ((ws0_sb, ws0_t, F32), (wn0_sb, wn0_t, F32),
                                      (ws1_sb, ws1b_t, BF16), (wn1_sb, wn1_t, F32)):
                    t = constp.tile([128, 128], dt_, name=f"w{len(lst)}_{id(src)}")
                    nc.sync.dma_start(t[:], src[e])
                    lst.append(t)
                for lst, src in ((b0_sb, b0_t), (b1_sb, b1_t),
                                 (gam_sb, gam_t), (bet_sb, bet_t)):
                    t = constp.tile([128, 1], F32, name=f"c{len(lst)}_{id(src)}")
                    nc.sync.dma_start(t[:], src[e])
                    lst.append(t)
            stats_sum = constp.tile([128, ne, nw], F32)
            stats_sq = constp.tile([128, ne, nw], F32)
            a_sb = constp.tile([128, ne], F32)
            c_sb = constp.tile([128, ne], F32)
            aw_sb = constp.tile([128, ne, 128], F32)   # a broadcast per expert
            cw_sb = constp.tile([128, ne, 128], F32)
            b1w_sb = constp.tile([128, ne, 128], F32)  # b1 broadcast per expert

            # ---- phase A: L0 gather-aggregate + dense + BN stats ----
            with (
                tc.tile_pool(name="gathA", bufs=2) as gathp,
                tc.tile_pool(name="idxA", bufs=8) as idxp,
                tc.tile_pool(name="dlA", bufs=2) as dlp,
                tc.tile_pool(name="sA", bufs=4) as sp,
                tc.tile_pool(name="aggA", bufs=4) as aggp,
                tc.tile_pool(name="xA", bufs=4) as xp,
                tc.tile_pool(name="h1A", bufs=6) as h1p,
                tc.tile_pool(name="wpsA", bufs=2, space="PSUM") as wpsp,
                tc.tile_pool(name="phA", bufs=4, space="PSUM") as php,
            ):
                for bi in range(nw):
                    wt = wpsp.tile([128, 512], F32, tag="wps",
                                   name=f"wpsA_{bi}")
                    nonzero = emit_agg(bi, (gathp, idxp, dlp, sp),
                                       xp_t, F32, [(0, 0, d)],
                                       lambda key, wt=wt: wt[:, 0:128],
                                       lambda key: 0, True, "A")
                    # feature-major mean aggregate for this window
                    aT = aggp.tile([128, 128], F32, tag="aggT",
                                   name=f"aTA_{bi}")
                    if nonzero:
                        nc.vector.tensor_copy(aT[:], wt[:, 0:128])
                    else:
                        nc.vector.memset(aT[:], 0.0)
                    xT = xp.tile([128, 128], F32, tag="xT", name=f"xTA_{bi}")
                    nc.sync.dma_start(
                        xT[:], xshfm_t[:, bi * 128:(bi + 1) * 128])
                    stage = h1p.tile([128, ne, 128], BF16, tag="h1",
                                     name=f"h1A_{bi}")
                    for e in range(ne):
                        hp = php.tile([128, 128], F32, tag="hp",
                                      name=f"hpA_{bi}_{e}")
                        nc.tensor.matmul(hp[:], ws0_sb[e][:], xT[:],
                                         start=True, stop=False)
                        nc.tensor.matmul(hp[:], wn0_sb[e][:], aT[:],
                                         start=False, stop=True)
                        sq = h1p.tile([128, 128], F32, tag="sq",
                                      name=f"sqA_{bi}_{e}")
                        nc.scalar.activation(
                            sq[:], hp[:], AF.Square,
                            accum_out=stats_sq[:, e, bi:bi + 1])
                        nc.vector.tensor_scalar(
                            stage[:, e, :], hp[:], b0_sb[e][:], None, OP.add,
                            OP.add, accum_out=stats_sum[:, e, bi:bi + 1])
                    nc.sync.dma_start(
                        h1s_t[bi].rearrange("e f n -> f e n"), stage[:])

            # ---- BN stats: reduce, AllReduce, finalize scale/bias ----
            with tc.tile_pool(name="bnf", bufs=1) as bnf:
                packed = bnf.tile([128, 2 * ne], F32)
                for e in range(ne):
                    nc.vector.reduce_sum(packed[:, e:e + 1],
                                         stats_sum[:, e, :], AX.X)
                    nc.vector.reduce_sum(packed[:, ne + e:ne + e + 1],
                                         stats_sq[:, e, :], AX.X)
                nc.sync.dma_start(stats_in_t[:], packed[:])
                nc.gpsimd.collective_compute(
                    "AllReduce", OP.add,
                    replica_groups=[list(range(n_cores))],
                    ins=[stats_in_t[:]], outs=[stats_out_t[:]])
                tot = bnf.tile([128, 2 * ne], F32)
                nc.sync.dma_start(tot[:], stats_out_t[:])
                inv_n = 1.0 / float(n_nodes)
                zeros = bnf.tile([128, 128], F32)
                nc.vector.memset(zeros[:], 0.0)
                for e in range(ne):
                    # stats_sum accumulated post-bias: all ntot rows (pads
                    # have hp=0) add exactly b0 each.
                    mean_pre = bnf.tile([128, 1], F32, name=f"mp{e}")
                    nc.scalar.mul(mean_pre[:], tot[:, e:e + 1], inv_n)
                    corr = float(ntot) * inv_n
                    mpre2 = bnf.tile([128, 1], F32, name=f"mp2_{e}")
                    nc.vector.tensor_scalar(mpre2[:], b0_sb[e][:], -corr,
                                            None, OP.mult)
                    nc.vector.tensor_scalar(mean_pre[:], mean_pre[:],
                                            mpre2[:], None, OP.add)
                    msq = bnf.tile([128, 1], F32, name=f"msq{e}")
                    nc.scalar.mul(msq[:], tot[:, ne + e:ne + e + 1], inv_n)
                    m2 = bnf.tile([128, 1], F32, name=f"m2_{e}")
                    nc.vector.tensor_scalar(m2[:], mean_pre[:], mean_pre[:],
                                            None, OP.mult)
                    var = bnf.tile([128, 1], F32, name=f"var{e}")
                    nc.vector.tensor_scalar(var[:], msq[:], m2[:], None,
                                            OP.subtract)
                    vare = bnf.tile([128, 1], F32, name=f"vare{e}")
                    nc.vector.tensor_scalar(vare[:], var[:], float(EPS),
                                            None, OP.add)
                    std = bnf.tile([128, 1], F32, name=f"std{e}")
                    nc.scalar.activation(std[:], vare[:], AF.Sqrt, bias=0.0)
                    rstd = bnf.tile([128, 1], F32, name=f"rstd{e}")
                    nc.vector.reciprocal(rstd[:], std[:])
                    nc.vector.tensor_scalar(a_sb[:, e:e + 1], gam_sb[e][:],
                                            rstd[:], None, OP.mult)
                    mean_post = bnf.tile([128, 1], F32, name=f"mpost{e}")
                    nc.vector.tensor_scalar(mean_post[:], mean_pre[:],
                                            b0_sb[e][:], None, OP.add)
                    ma = bnf.tile([128, 1], F32, name=f"ma{e}")
                    nc.vector.tensor_scalar(ma[:], mean_post[:],
                                            a_sb[:, e:e + 1], None, OP.mult)
                    nc.vector.tensor_scalar(c_sb[:, e:e + 1], bet_sb[e][:],
                                            ma[:], None, OP.subtract)
                    # broadcast a, c, b1 across a 128-wide tile for batched
                    # tensor-tensor ops in phases B and C
                    nc.vector.tensor_scalar(aw_sb[:, e, :], zeros[:],
                                            a_sb[:, e:e + 1], None, OP.add)
                    nc.vector.tensor_scalar(cw_sb[:, e, :], zeros[:],
                                            c_sb[:, e:e + 1], None, OP.add)
                    nc.vector.tensor_scalar(b1w_sb[:, e, :], zeros[:],
                                            b1_sb[e][:], None, OP.add)

            # ---- phase B: BN apply + relu; node-major 2KB rows ----
            with (
                tc.tile_pool(name="pb", bufs=3) as pb,
                tc.tile_pool(name="stg", bufs=3) as stgp,
                tc.tile_pool(name="pbps", bufs=2, space="PSUM") as pbps,
            ):
                for w in range(nw):
                    htw = pb.tile([128, ne, 128], BF16, tag="ht",
                                  name=f"bht{w}")
                    nc.sync.dma_start(
                        htw[:], h1s_t[w].rearrange("e f n -> f e n"))
                    t1 = pb.tile([128, ne, 128], F32, tag="t1",
                                 name=f"bt1_{w}")
                    nc.vector.scalar_tensor_tensor(
                        t1[:], htw[:], 1.0, aw_sb[:], OP.mult, OP.mult)
                    ab = pb.tile([128, ne, 128], BF16, tag="ab",
                                 name=f"bab{w}")
                    nc.vector.scalar_tensor_tensor(
                        ab[:], t1[:], 1.0, cw_sb[:], OP.mult, OP.add)
                    rl = pb.tile([128, ne, 128], BF16, tag="rl",
                                 name=f"brl{w}")
                    nc.scalar.activation(rl[:], ab[:], AF.Relu)
                    nc.sync.dma_start(
                        h1afm_t[w].rearrange("e f n -> f e n"), rl[:])
                    pt = pbps.tile([128, ne * 128], BF16, tag="bpt",
                                   name=f"bpt{w}")
                    for e in range(ne):
                        nc.tensor.transpose(pt[:, e * 128:(e + 1) * 128],
                                            rl[:, e, :], identb_sb[:])
                    stage = stgp.tile([128, de], BF16, tag="stage",
                                      name=f"stg_{w}")
                    nc.vector.tensor_copy(stage[:], pt[:])
                    nc.sync.dma_start(
                        h1loc_t[w * 128:(w + 1) * 128, :], stage[:])

            # ---- AllGather h1 across cores ----
            if not os.environ.get("KERNEL_SKIP_AG"):
                with tc.tile_pool(name="agp", bufs=1):
                    nc.gpsimd.collective_compute(
                        "AllGather", OP.bypass,
                        replica_groups=[list(range(n_cores))],
                        ins=[h1loc_t[:]], outs=[h1all_t[:]])

            # ---- phase C: L1 gather-aggregate + dense -> out ----
            with (
                tc.tile_pool(name="gathC", bufs=2) as gathp,
                tc.tile_pool(name="idxC", bufs=8) as idxp,
                tc.tile_pool(name="dlC", bufs=2) as dlp,
                tc.tile_pool(name="sC", bufs=4) as sp,
                tc.tile_pool(name="aggC", bufs=4) as aggp,
                tc.tile_pool(name="hC", bufs=4) as hcp,
                tc.tile_pool(name="h2C", bufs=4) as h2p,
                tc.tile_pool(name="wpsC", bufs=2, space="PSUM") as wpsp,
                tc.tile_pool(name="ptC", bufs=2, space="PSUM") as pstp,
                tc.tile_pool(name="phC", bufs=2, space="PSUM") as php,
            ):
                for bi in range(nw):
                    wts = [wpsp.tile([128, 512], F32, tag=f"wq{q}",
                                     name=f"wpsC_{bi}_{q}")
                           for q in range(neq)]
                    nonzero = emit_agg(
                        bi, (gathp, idxp, dlp, sp), h1all_t, BF16,
                        [(q, q * 512, 512) for q in range(neq)],
                        lambda q, wts=wts: wts[q][:, :],
                        lambda q: q, False, "C")
                    for q in range(neq):
                        # node-major 4-expert mean aggregate -> sbuf
                        aw4 = aggp.tile([128, 512], F32, tag="agg",
                                        name=f"aw4C_{bi}_{q}")
                        if nonzero:
                            nc.vector.tensor_copy(aw4[:], wts[q][:, :])
                        else:
                            nc.vector.memset(aw4[:], 0.0)
                        pt4 = pstp.tile([128, 512], F32, tag="pt",
                                        name=f"pt4C_{bi}_{q}")
                        for j in range(4):
                            nc.tensor.transpose(pt4[:, j * 128:(j + 1) * 128],
                                                aw4[:, j * 128:(j + 1) * 128],
                                                ident_sb[:])
                        aT4 = aggp.tile([128, 512], F32, tag="aggT",
                                        name=f"aT4C_{bi}_{q}")
                        nc.vector.tensor_copy(aT4[:], pt4[:])
                        hfm4 = hcp.tile([128, 4, 128], BF16, tag="hfm",
                                        name=f"hfmC_{bi}_{q}")
                        nc.sync.dma_start(
                            hfm4[:], h1afm_t[bi, q * 4:q * 4 + 4].rearrange(
                                "e f n -> f e n"))
                        hp2 = php.tile([128, 4, 128], F32, tag="hp2",
                                       name=f"hp2C_{bi}_{q}")
                        for j in range(4):
                            nc.tensor.matmul(
                                hp2[:, j, :],
                                ws1_sb[q * 4 + j][:], hfm4[:, j, :],
                                start=j == 0, stop=False)
                            nc.tensor.matmul(
                                hp2[:, j, :],
                                wn1_sb[q * 4 + j][:],
                                aT4[:, j * 128:(j + 1) * 128],
                                start=False, stop=True)
                        h2 = h2p.tile([128, 4, 128], F32, tag="h2",
                                      name=f"h2C_{bi}_{q}")
                        nc.vector.scalar_tensor_tensor(
                            h2[:], hp2[:], 1.0, b1w_sb[:, q * 4:q * 4 + 4, :],
                            OP.mult, OP.add)
                        nc.sync.dma_start(
                            out_t[bi, q * 4:q * 4 + 4].rearrange(
                                "e f n -> f e n"), h2[:])

    nc.compile()
    return nc


# --------------------------------------------------------------------------
# input map assembly + public entry point
# --------------------------------------------------------------------------
def make_in_maps(meta, x, W_self, W_nbr, b, gamma, beta, n_cores=N_CORES):
    n_nodes, d = x.shape
    ntot, shard = meta["ntot"], meta["shard"]
    xp = np.zeros((ntot, d), np.float32)
    xp[:n_nodes] = x
    iota_np = np.tile(np.arange(128, dtype=np.float32)[None, :], (128, 1))
    common = {
        "xp": xp,
        "iota": iota_np,
        "ident": np.eye(128, dtype=np.float32),
        "identb": np.eye(128, dtype=ml_dtypes.bfloat16),
        "Ws0": W_self[:, 0].astype(np.float32),
        "Wn0": W_nbr[:, 0].astype(np.float32),
        "Ws1b": W_self[:, 1].astype(ml_dtypes.bfloat16),
        "Wn1": W_nbr[:, 1].astype(np.float32),
        "b0": b[:, 0][:, :, None].astype(np.float32),
        "b1": b[:, 1][:, :, None].astype(np.float32),
        "gamma": gamma[:, 0][:, :, None].astype(np.float32),
        "beta": beta[:, 0][:, :, None].astype(np.float32),
    }
    in_maps = []
    for c in range(n_cores):
        m = dict(common)
        m["xshfm"] = np.ascontiguousarray(xp[c * shard:(c + 1) * shard].T)
        m["idx"] = meta["idx_cores"][c]
        m["dl"] = meta["dl_cores"][c]
        m["cnt"] = meta["cnt_cores"][c]
        in_maps.append(m)
    return in_maps


def assemble_out(meta, results, n_cores=N_CORES):
    n_nodes = meta["n_nodes"]
    # per core [nw, ne, d, 128] -> [node, feat, expert]
    big = np.stack([np.asarray(results[c]["out"]) for c in range(n_cores)])
    big = big.transpose(0, 1, 4, 3, 2)      # [c, w, n, f, e]
    big = big.reshape(meta["ntot"], D, N_EXPERTS)
    return np.ascontiguousarray(big[:n_nodes])


def kernel(x, edge_index, W_self, W_nbr, b, gamma, beta):
    x = np.asarray(x, dtype=np.float32)
    edge_index = np.asarray(edge_index)
    W_self = np.asarray(W_self, dtype=np.float32)
    W_nbr = np.asarray(W_nbr, dtype=np.float32)
    b = np.asarray(b, dtype=np.float32)
    gamma = np.asarray(gamma, dtype=np.float32)
    beta = np.asarray(beta, dtype=np.float32)

    meta = preprocess(edge_index, x.shape[0])
    nc = build_program(meta)
    in_maps = make_in_maps(meta, x, W_self, W_nbr, b, gamma, beta)
    res = run_bass_kernel_spmd(nc, in_maps, list(range(N_CORES)))
    return assemble_out(meta, res.results)
